# revision 6
# baseline (speedup 1.0000x reference)
"""Trainium2 Bass kernel for nn_CNNRNN_63625645523427.

Model: relu-gated LSTM decoder over label embeddings with per-step class
prediction.

  proj_img = img @ proj_I; x0 = relu(proj_img); pred0 = x0 @ U_l.T
  per step t:  gates = relu(lab_t @ W_ih.T + b_ih + h @ W_hh.T + b_hh)
               c = f*c + i*cg;  h = o * relu(c)
               x_t = relu(proj_img + h @ proj_O);  pred_t = x_t @ U_l.T

Sharding (8 cores): the recurrence is replicated on every core with the full
batch B=128 (PE matmul cost is independent of M<=128, so batch-sharding the
recurrence saves nothing, and per-step collectives have a ~5us floor).  The
large prediction matmul [128,512]@[512,7178] is sharded over the class dim C:
each core computes a 898-wide slice of the logits (7184 = 8*898, padded from
7178).  Host gathers/concats the per-core slices.

Matmul operands are fp16 (PE streams 1 col/cycle -> 2x faster than
fp32/f32r which run at 2 cycles/col; fast-weight-load applies; fp16's
10-bit mantissa is ~4x more accurate than bf16 and all values here are
far inside fp16 range).  Accumulation stays fp32 in PSUM; the cell state
c and gate activations are kept fp32.

PE stream is software-pipelined so the tensor engine never waits on the
recurrence's elementwise chain:
  h-MMs(t) | lab-MMs(t+1) | nhT-transpose(t) | pred-lo(t-1) | x-MMs(t)
  | pred-hi(t-1) | xT-transpose(t) | h-MMs(t+1) ...
"""

import os
from contextlib import ExitStack

import numpy as np

import concourse.bass as bass
import concourse.tile as tile
import concourse.mybir as mybir
from concourse import bacc
from concourse.bass_utils import run_bass_kernel_spmd
from concourse.masks import make_identity

B = 128          # batch
T = 20           # labels per sample (output steps)
NS = T - 1       # recurrent steps
L = 512          # feature width (n_feats_lstm == n_feats_conv)
C = 7178         # num classes
G = 4 * L        # gates width
NCORES = 8
CS = 898         # per-core class shard (8*898 = 7184 >= 7178)
KL = L // 128    # K chunks for a 512 contraction

F32 = mybir.dt.float32
F16 = mybir.dt.float16
NPF16 = np.float16
RELU = mybir.ActivationFunctionType.Relu

_CACHED_NC = None

# Populated by kernel() with the last BassKernelResults (exec_time_ns etc.).
LAST_RESULT = None


def _build():
    """Build + schedule the single-core Bass program (identical on all cores;
    per-core data differs only in the U_l.T shard)."""
    nc = bacc.Bacc("TRN2", target_bir_lowering=False, debug=False,
                   num_devices=NCORES)

    # All inputs are pre-arranged on the host to [128, k, n] partition-major
    # layouts so every DMA is a contiguous burst (no strided packets).
    d_wcat = nc.dram_tensor("wcat", [128, 2 * KL, G], F16,
                            kind="ExternalInput")
    d_projO = nc.dram_tensor("projO", [128, KL, L], F16, kind="ExternalInput")
    d_projI = nc.dram_tensor("projI", [128, KL, L], F16, kind="ExternalInput")
    d_imgT = nc.dram_tensor("imgT", [128, KL, B], F16, kind="ExternalInput")
    d_labT = nc.dram_tensor("labT", [NS, 128, KL, B], F16,
                            kind="ExternalInput")
    d_ulT = nc.dram_tensor("ulT", [128, KL, CS], F16, kind="ExternalInput")
    d_h0T = nc.dram_tensor("h0T", [128, KL, B], F16, kind="ExternalInput")
    d_bsum = nc.dram_tensor("bsum", [B, G], F32, kind="ExternalInput")
    d_c0 = nc.dram_tensor("c0b", [B, L], F32, kind="ExternalInput")
    d_out = nc.dram_tensor("preds", [T, B, CS], F32, kind="ExternalOutput")

    with tile.TileContext(nc) as tc, ExitStack() as ctx:
        consts = ctx.enter_context(tc.tile_pool(name="consts", bufs=1))
        labp = ctx.enter_context(tc.tile_pool(name="lab", bufs=3))
        act = ctx.enter_context(tc.tile_pool(name="act", bufs=2))
        gatep = ctx.enter_context(tc.tile_pool(name="gate", bufs=3))
        predp = ctx.enter_context(tc.tile_pool(name="pred", bufs=3))
        psum = ctx.enter_context(tc.tile_pool(name="ps", bufs=6, space="PSUM"))
        psum_p = ctx.enter_context(
            tc.tile_pool(name="psp", bufs=2, space="PSUM"))

        # --- constants / weights (DMA order = need order) ------------------
        ident = consts.tile([128, 128], F16, tag="ident")
        make_identity(nc, ident[:])
        projI_sb = consts.tile([128, KL, L], F16, tag="projI")
        nc.sync.dma_start(projI_sb[:], d_projI.ap())
        imgT_sb = consts.tile([128, KL, B], F16, tag="imgT")
        nc.sync.dma_start(imgT_sb[:], d_imgT.ap())
        hT = act.tile([128, KL, B], F16, tag="hT")
        nc.sync.dma_start(hT[:], d_h0T.ap())
        wcat_sb = consts.tile([128, 2 * KL, G], F16, tag="wcat")
        nc.sync.dma_start(wcat_sb[:], d_wcat.ap())
        ulT_sb = consts.tile([128, KL, CS], F16, tag="ulT")
        nc.sync.dma_start(ulT_sb[:], d_ulT.ap())
        projO_sb = consts.tile([128, KL, L], F16, tag="projO")
        nc.sync.dma_start(projO_sb[:], d_projO.ap())
        bsum_sb = consts.tile([128, G], F32, tag="bsum")
        nc.sync.dma_start(bsum_sb[:], d_bsum.ap())
        c_prev = act.tile([128, L], F32, tag="c")
        nc.sync.dma_start(c_prev[:], d_c0.ap())

        def transpose_to(src_sb, tag):
            """[128, L] bf16 -> [128, KL, 128] bf16 (transposed chunks)."""
            tp = psum.tile([128, KL, 128], F16, tag="ps")
            for k in range(KL):
                nc.tensor.transpose(
                    tp[:, k, :], src_sb[:, 128 * k:128 * (k + 1)], ident[:])
            dst = act.tile([128, KL, B], F16, tag=tag)
            nc.vector.tensor_copy(dst[:], tp[:])
            return dst

        def lab_mms(t):
            """Open 4 gates psum tiles; bias (K=1 ones row) + lab-part MMs."""
            lab_sb = labp.tile([128, KL, B], F16, tag="lab")
            nc.sync.dma_start(lab_sb[:], d_labT.ap()[t - 1])
            tiles = []
            for n in range(4):
                gps = psum.tile([128, 512], F32, tag="ps")
                nsl = slice(512 * n, 512 * (n + 1))
                for k in range(KL):
                    nc.tensor.matmul(gps[:], lab_sb[:, k, :],
                                     wcat_sb[:, k, nsl],
                                     start=(k == 0), stop=False)
                tiles.append(gps)
            return tiles

        def h_mms(gtiles, hT_):
            """Accumulate the h-part into the open gates psum tiles."""
            for n in range(4):
                nsl = slice(512 * n, 512 * (n + 1))
                for k in range(KL):
                    nc.tensor.matmul(gtiles[n][:], hT_[:, k, :],
                                     wcat_sb[:, KL + k, nsl],
                                     start=False, stop=(k == KL - 1))

        def gates_elemwise(gtiles, c_prev):
            """gates = relu(psum + bias); cell update; (nh, c_new)."""
            g_relu = []
            for n in range(4):
                nsl = slice(512 * n, 512 * (n + 1))
                gpre = gatep.tile([128, 512], F32, tag="gpre")
                nc.vector.tensor_add(gpre[:], gtiles[n][:], bsum_sb[:, nsl])
                gr = gatep.tile([128, 512], F32, tag=f"grelu{n}")
                nc.scalar.activation(gr[:], gpre[:], RELU)
                g_relu.append(gr)
            i_g, f_g, cg_g, o_g = g_relu
            t1 = act.tile([128, L], F32, tag="t1")
            nc.vector.tensor_mul(t1[:], f_g[:], c_prev[:])
            t2 = act.tile([128, L], F32, tag="t2")
            nc.vector.tensor_mul(t2[:], i_g[:], cg_g[:])
            c_new = act.tile([128, L], F32, tag="c")
            nc.vector.tensor_add(c_new[:], t1[:], t2[:])
            rc = act.tile([128, L], F32, tag="rc")
            nc.scalar.activation(rc[:], c_new[:], RELU)
            nh = act.tile([128, L], F16, tag="nh")
            nc.vector.tensor_mul(nh[:], o_g[:], rc[:])
            return nh, c_new

        def pred_lo(xT_sb):
            ps1 = psum_p.tile([128, 512], F32, tag="psp")
            for k in range(KL):
                nc.tensor.matmul(ps1[:], xT_sb[:, k, :], ulT_sb[:, k, 0:512],
                                 start=(k == 0), stop=(k == KL - 1))
            return ps1

        def pred_hi_and_store(ps1, xT_sb, t):
            ps2 = psum_p.tile([128, CS - 512], F32, tag="psp")
            for k in range(KL):
                nc.tensor.matmul(ps2[:], xT_sb[:, k, :], ulT_sb[:, k, 512:CS],
                                 start=(k == 0), stop=(k == KL - 1))
            pred_sb = predp.tile([128, CS], F32, tag="pred")
            nc.vector.tensor_copy(pred_sb[:, 0:512], ps1[:])
            nc.vector.tensor_copy(pred_sb[:, 512:CS], ps2[:])
            nc.sync.dma_start(d_out.ap()[t], pred_sb[:])

        def x_step(hT_):
            """x = relu(proj_img + h @ proj_O), relu+cast to fp16 on ACT."""
            xps = psum_p.tile([128, L], F32, tag="psp")
            for k in range(KL):
                nc.tensor.matmul(xps[:], hT_[:, k, :], projO_sb[:, k, :],
                                 start=(k == 0), stop=(k == KL - 1))
            xpre = act.tile([128, L], F32, tag="xpre")
            nc.vector.tensor_add(xpre[:], xps[:], proj_img[:])
            x_sb = act.tile([128, L], F16, tag="x")
            nc.scalar.activation(x_sb[:], xpre[:], RELU)
            return x_sb

        # --- phase 0: proj_img, x0 = relu(proj_img) ------------------------
        pi_ps = psum.tile([128, L], F32, tag="ps")
        for k in range(KL):
            nc.tensor.matmul(pi_ps[:], imgT_sb[:, k, :], projI_sb[:, k, :],
                             start=(k == 0), stop=(k == KL - 1))
        proj_img = consts.tile([128, L], F32, tag="projimg")
        nc.vector.tensor_copy(proj_img[:], pi_ps[:])
        x_prev = act.tile([128, L], F16, tag="x")
        nc.scalar.activation(x_prev[:], pi_ps[:], RELU)

        # --- software-pipelined main loop ----------------------------------
        gtiles = lab_mms(1)
        for t in range(1, NS + 1):
            h_mms(gtiles, hT)
            xT = transpose_to(x_prev, "xT")       # x(t-1) transposed
            cur_gtiles = gtiles
            if t < NS:
                gtiles = lab_mms(t + 1)
            nh, c_prev = gates_elemwise(cur_gtiles, c_prev)
            hT_new = transpose_to(nh, "hT")
            ps1 = pred_lo(xT)                     # pred of step t-1
            pred_hi_and_store(ps1, xT, t - 1)
            x_prev = x_step(hT_new)
            hT = hT_new

        # final prediction (step NS)
        xT = transpose_to(x_prev, "xT")
        ps1 = pred_lo(xT)
        pred_hi_and_store(ps1, xT, NS)

    nc.compile()
    return nc


def kernel(img_embeddings, labels_idx, U_l, proj_I, proj_O,
           W_ih, b_ih, W_hh, b_hh, h0, c0):
    global _CACHED_NC, LAST_RESULT
    img = np.asarray(img_embeddings, np.float32)
    idx = np.asarray(labels_idx)
    U_l = np.asarray(U_l, np.float32)
    proj_I = np.asarray(proj_I, np.float32)
    proj_O = np.asarray(proj_O, np.float32)
    W_ih = np.asarray(W_ih, np.float32)
    W_hh = np.asarray(W_hh, np.float32)
    b_ih = np.asarray(b_ih, np.float32)
    b_hh = np.asarray(b_hh, np.float32)
    h0 = np.asarray(h0, np.float32)
    c0 = np.asarray(c0, np.float32)

    def bf(x):
        return np.ascontiguousarray(x.astype(NPF16))

    def pkn(x):
        # [k*128, n] -> [128, k, n] partition-major for contiguous DMA
        kk = x.shape[0] // 128
        return np.ascontiguousarray(
            x.reshape(kk, 128, x.shape[1]).transpose(1, 0, 2))

    # Host-side sharding / layout prep (transposes + row gather of U_l).
    lab = U_l[idx[:, :NS]]                                   # [B, NS, L]
    labT = lab.transpose(1, 2, 0)                            # [NS, L, B]
    labT = bf(np.stack([pkn(labT[t]) for t in range(NS)]))   # [NS,128,KL,B]
    wcat = bf(pkn(np.concatenate([W_ih.T, W_hh.T], axis=0)))
    bsum = np.ascontiguousarray(
        np.broadcast_to((b_ih + b_hh)[None, :], (B, G)))     # [B, G]
    imgT = bf(pkn(img.T))
    h0T = bf(pkn(np.broadcast_to(h0[:, None], (L, B))))
    c0b = np.ascontiguousarray(np.broadcast_to(c0[None, :], (B, L)))
    ulT = np.zeros((L, NCORES * CS), np.float32)
    ulT[:, :C] = U_l.T

    if _CACHED_NC is None:
        _CACHED_NC = _build()
    nc = _CACHED_NC

    common = {
        "wcat": wcat, "projO": bf(pkn(proj_O)), "projI": bf(pkn(proj_I)),
        "imgT": imgT, "labT": labT, "h0T": h0T, "bsum": bsum, "c0b": c0b,
    }
    in_maps = [
        dict(common, ulT=bf(pkn(ulT[:, c * CS:(c + 1) * CS])))
        for c in range(NCORES)
    ]

    res = run_bass_kernel_spmd(nc, in_maps, core_ids=list(range(NCORES)))
    LAST_RESULT = res
    if res.exec_time_ns is not None:
        print(f"HW exec time: {res.exec_time_ns} ns")

    allpred = np.concatenate(
        [res.results[c]["preds"] for c in range(NCORES)], axis=2)
    out = np.ascontiguousarray(allpred[:, :, :C].transpose(1, 0, 2))
    return out
